# revision 40
# baseline (speedup 1.0000x reference)
"""L2-distance attention (B=4, DIM=512, N=2048, H=8, D=32) on 8 trn2 NeuronCores.

Sharding: core c handles batch b = c//2, query-half = c%2 (1024 queries, all
2048 keys, all 8 heads).  Output is a pure concat - no cross-core reduce.

Design (vs the 382us fp32r baseline; measured ~326-333us):
  * fp16 inputs/projections (host-side cast halves DMA; PE still 1 cyc/col).
  * dist2 via fp8e4m3 *DoubleRow* matmuls (0.5 cyc/col) with full error
    compensation: DoubleRow contraction rows are free, so the q/k fp8
    rounding residuals ride extra row-groups (m8,q8)+(m8,qres)+(mres,q8)
    and a 3-term fp8 split of the q2 row - end-to-end error stays at the
    fp16 level (8.4e-4).  k2[j] is per-PSUM-partition, so it rides the
    ACT sqrt *bias port* exactly (computed by 1-column transpose matmuls).
    Per-head DoubleRow operand tiles are DMA-gathered from whole-tensor
    fp8 staging tiles; the tensor engine never pays for the layout.
  * ACT does ONLY the sqrt pass (w = (c/4)*sqrt(d2+delta), c=scale*log2 e),
    one table load total.  The exp pass moved off ACT entirely:
  * exp via ONE custom-DVE instruction per chunk (registered at build
    time): a minimax cubic p(w) of 2^-w on [0, 0.95], squared twice in
    the 8-stage DVE ALU pipeline: E = p(w)^4 = 2^(-4w) = exp(-scale*dist),
    rel err 2.4e-4, fp16 in/out, in-place.
  * PSUM in 8 banks: pd2 2x[128,1024]f32 + po 2x[33,1024]f32; the
    k2/q2/prep needs borrow dist2 pool slots.
  * Remaining wall-clock is set by a hardware PE duty-cycle governor
    (3.4us at 8/8 then 30.7us at 4/8 under sustained matmul load) and by
    the cost of fp8<->fp16 weight-mode switches (~0.4-0.7us each), which
    rules out finer dist2/attnv interleaving (measured: 4-jt groups
    +12us, 2-mm alternation +80us, half-jt groups +22us).
"""

import numpy as np

import concourse.bass as bass
import concourse.mybir as mybir
import concourse.tile as tile
from concourse import bacc

F32 = mybir.dt.float32
F16 = mybir.dt.float16
F8 = mybir.dt.float8e4
AF = mybir.ActivationFunctionType
ALU = mybir.AluOpType
DR = mybir.MatmulPerfMode.DoubleRow

B, DIM, N = 4, 512, 2048
H, D = 8, 32
INNER = H * D            # 256
NQ = N // 2              # 1024 queries per core
P = 128
KT = DIM // P            # 4 contraction tiles for the projections
NJT = N // P             # 16 key tiles
VTW = D + 1              # 33: v columns + ones column per head
VSTRIDE = H * VTW        # 264 columns per key-tile block of vt
SCALE = float(D) ** -0.5
DELTA = 0.05             # dist2 guard against fp16 rounding (sqrt>0)
C_EXP = SCALE * float(np.log2(np.e))   # u = C_EXP * dist; E = 2^-u
S4 = (C_EXP * C_EXP) / 16.0            # sqrt scale so ACT emits w = u/4

# minimax cubic for 2^-w on [0, 0.95] (rel err 5.9e-5; E=p^4 -> 2.4e-4)
A3, A2, A1, A0 = -0.0398455, 0.23114166, -0.69136122, 0.99994121


def _register_exp_op():
    """Register the quartic-exp custom DVE op (idempotent).

    body: sq(sq(((a3*w + a2)*w + a1)*w + a0)) = p(w)^4 ~= 2^(-4w).
    a3,a2 ride C0,C1; a1 rides imm2 (compile-time literal); a0 rides the
    spilled C3 (delivered via in1 as a [P,1] broadcast).
    """
    from concourse import dve_ops as dops
    from concourse.dve_spec import (
        C0, C1, C2, C3, Spec, Src0, _has_src1, _spill_c3_to_src1, lower, sq,
    )
    from concourse.dve_uop import DveOpSpec

    NAME = "EXPQ4_ANT"
    for op in dops.OPS:
        if op.name == NAME:
            return op

    w = Src0
    p = ((C0 * w + C1) * w + C2) * w + C3
    body = _spill_c3_to_src1(sq(sq(p)))
    spec = Spec(
        body=body,
        reference=lambda in0, in1, s0, s1, imm2: (
            (((s0 * in0 + s1) * in0 + imm2) * in0 + in1) ** 4
        ),
    )
    row = dops._CUSTOM_DVE_ROW_BASE + len(dops.OPS)
    shas = {}
    for ver in ("v3", "v4"):
        tmp = DveOpSpec(name=NAME, opcode=row, uops=lower(spec, ver=ver),
                        rd1_en=_has_src1(spec))
        shas[ver] = tmp.sha(ver)
    op = dops.DveOp(NAME, spec, subdim=False, uops_sha=shas)
    dops.OPS.append(op)
    dops._SUB_OPCODE_FOR_NAME[NAME] = row
    dops.CUSTOM_DVE_SPECS[NAME] = spec
    return op


def make_mm(nc):
    def mm(out, lhsT, rhs, start, stop):
        nc.tensor.matmul(out, lhsT, rhs, start=start, stop=stop)
    return mm


def build_program() -> bass.Bass:
    exp_op = _register_exp_op()
    nc = bacc.Bacc("TRN2", target_bir_lowering=False, debug=False)

    xq_d = nc.declare_dram_parameter("xq", [DIM, NQ], F16, isOutput=False)
    xkv_d = nc.declare_dram_parameter("xkv", [DIM, N], F16, isOutput=False)
    wq_d = nc.declare_dram_parameter("wq", [DIM, INNER], F16, isOutput=False)
    wkv_d = nc.declare_dram_parameter("wkv", [DIM, 2 * INNER], F16, isOutput=False)
    wo_d = nc.declare_dram_parameter("wo", [INNER, DIM], F16, isOutput=False)
    b_d = nc.declare_dram_parameter("b", [DIM], F32, isOutput=False)
    z_d = nc.declare_dram_parameter("z", [DIM, NQ], F32, isOutput=True)

    with tile.TileContext(nc) as tc:
        mm = make_mm(nc)
        with tc.tile_pool(name="keep", bufs=1) as keep, \
             tc.tile_pool(name="work", bufs=2) as work:

            # ---- persistent tiles ----
            q_t = [keep.tile([P, NQ], F16, tag=f"q{m}", name=f"q{m}") for m in range(2)]
            k_t = [keep.tile([P, N], F16, tag=f"k{m}", name=f"k{m}") for m in range(2)]
            vt_big = keep.tile([P, NJT * VSTRIDE], F16, tag="vt", name="vt")
            y_t = [keep.tile([P, NQ], F16, tag=f"y{m}", name=f"y{m}") for m in range(2)]
            wo_t = [keep.tile([P, DIM], F16, tag=f"wo{m}", name=f"wo{m}") for m in range(2)]
            b_t = keep.tile([P, KT], F32, tag="bias", name="bias")
            ones16 = keep.tile([64, 32], F16, tag="ones16", name="ones16")
            onesP16 = keep.tile([P, 1], F16, tag="onesP16", name="onesP16")
            a0_t = keep.tile([P, 1], F32, tag="a0c", name="a0c")

            nc.vector.memset(ones16[:, :], 1.0)
            nc.vector.memset(onesP16[:, :], 1.0)
            # masked ones: col g = 1.0 on partitions [32g, 32g+32), else 0
            ones4 = keep.tile([P, 4], F16, tag="ones4", name="ones4")
            nc.vector.memset(ones4[:, :], 0.0)
            for g in range(4):
                nc.vector.memset(ones4[g * 32:(g + 1) * 32, g:g + 1], 1.0)
            nc.vector.memset(a0_t[:, :], A0)

            # ======== Phase A: projections ========
            with tc.tile_pool(name="xw", bufs=1) as xw, \
                 tc.tile_pool(name="pp", bufs=3, space="PSUM") as pp:
                wq_t = [xw.tile([P, INNER], F16, tag=f"wq{k}", name=f"wq{k}") for k in range(KT)]
                wkv_t = [xw.tile([P, 2 * INNER], F16, tag=f"wkv{k}", name=f"wkv{k}") for k in range(KT)]
                xq_t = [xw.tile([P, NQ], F16, tag=f"xq{k}", name=f"xq{k}") for k in range(KT)]
                xkv_t = [xw.tile([P, N], F16, tag=f"xkv{k}", name=f"xkv{k}") for k in range(KT)]

                xq_r = xq_d[:].rearrange("(t p) n -> t p n", p=P)
                xkv_r = xkv_d[:].rearrange("(t p) n -> t p n", p=P)
                wq_r = wq_d[:].rearrange("(t p) o -> t p o", p=P)
                wkv_r = wkv_d[:].rearrange("(t p) o -> t p o", p=P)
                for k in range(KT):
                    nc.sync.dma_start(out=xq_t[k][:, :], in_=xq_r[k])
                    nc.sync.dma_start(out=xkv_t[k][:, :], in_=xkv_r[k])
                    nc.sync.dma_start(out=wq_t[k][:, :], in_=wq_r[k])
                    nc.sync.dma_start(out=wkv_t[k][:, :], in_=wkv_r[k])
                wo_r = wo_d[:].rearrange("(t p) o -> t p o", p=P)
                for m in range(2):
                    nc.sync.dma_start(out=wo_t[m][:, :], in_=wo_r[m])
                nc.sync.dma_start(out=b_t[:, :], in_=b_d[:].rearrange("(t p) -> p t", p=P))

                # q projection: (DIM x NQ) -> (INNER x NQ)
                for m in range(2):
                    for n in range(NQ // 512):
                        ps = pp.tile([P, 512], F32, tag="proj", name="proj")
                        for k in range(KT):
                            mm(ps[:, :],
                               wq_t[k][:, m * P:(m + 1) * P],
                               xq_t[k][:, n * 512:(n + 1) * 512],
                               start=(k == 0), stop=(k == KT - 1))
                        nc.vector.tensor_copy(q_t[m][:, n * 512:(n + 1) * 512], ps[:, :])

                # k projection: (DIM x N) -> (INNER x N)   (wkv cols 0:256)
                # PSUM->SBUF copies go through ACT (identity, no table cost);
                # DVE is the tighter engine.
                for m in range(2):
                    for n in range(N // 512):
                        ps = pp.tile([P, 512], F32, tag="proj", name="proj")
                        for k in range(KT):
                            mm(ps[:, :],
                               wkv_t[k][:, m * P:(m + 1) * P],
                               xkv_t[k][:, n * 512:(n + 1) * 512],
                               start=(k == 0), stop=(k == KT - 1))
                        nc.scalar.activation(k_t[m][:, n * 512:(n + 1) * 512],
                                             ps[:, :], AF.Identity, scale=1.0)

                # v^T projection: per key tile jt, (128 j x 256 d), strided into
                # vt_big so each head's 32 columns sit next to its ones column.
                nc.vector.tensor_copy(
                    vt_big[:, :].rearrange("p (a c) -> p a c", c=VTW)[:, :, D:D + 1],
                    onesP16[:, 0:1].to_broadcast((P, P, 1)))
                for jt in range(NJT):
                    ps = pp.tile([P, INNER], F32, tag="vtps", name="vtps")
                    for k in range(KT):
                        mm(ps[:, :],
                           xkv_t[k][:, jt * P:(jt + 1) * P],
                           wkv_t[k][:, INNER:2 * INNER],
                           start=(k == 0), stop=(k == KT - 1))
                    dst = vt_big[:, jt * VSTRIDE:(jt + 1) * VSTRIDE] \
                        .rearrange("p (h c) -> p h c", c=VTW)[:, :, 0:D]
                    src = ps[:, :].rearrange("p (h d) -> p h d", d=D)
                    nc.scalar.activation(dst, src, AF.Identity, scale=1.0)

            # ======== Phase B: attention heads ========
            # dist2 via fp8e4m3 DoubleRow matmuls (0.5 cyc/col) with full
            # error compensation: extra contraction rows are free, so the
            # logical 100-row contraction is
            #   rows  0:32  (m8[d],  q8[d])     m8 = fp8(-2k), q8 = fp8(q)
            #   rows 32:64  (m8[d],  qres[d])   qres = fp8(q - q8)
            #   rows 64:96  (mres[d],q8[d])     mres = fp8(-2k - m8)
            #   rows 96:99  (1, c0/c1f/c2f)     3-term fp8 split of q2
            #   row  99     (0, 0)
            # mapped to 50 DoubleRow partition-pairs: r = half*50 + p.
            # k2[j] stays exact on the ACT sqrt bias port (per-partition).
            with tc.tile_pool(name="pd2", bufs=2, space="PSUM") as pd2, \
                 tc.tile_pool(name="po", bufs=2, space="PSUM") as po:
                KP = 50
                kt8_t = [keep.tile([KP, 2 * N], F8, tag=f"kt8{i}", name=f"kt8{i}")
                         for i in range(2)]
                qt8_t = [keep.tile([KP, 2 * NQ], F8, tag=f"qt8{i}", name=f"qt8{i}")
                         for i in range(2)]
                eq_t = [keep.tile([P, NJT * NQ], F16, tag=f"eq{i}", name=f"eq{i}")
                        for i in range(2)]
                po_s = [keep.tile([VTW, NQ], F32, tag=f"pos{i}", name=f"pos{i}")
                        for i in range(2)]
                # fp8 staging (whole-tensor ops; per-head slices DMA-gathered)
                m8_a = [keep.tile([P, N], F8, tag=f"m8{m}", name=f"m8{m}") for m in range(2)]
                mres_a = [keep.tile([P, N], F8, tag=f"mr{m}", name=f"mr{m}") for m in range(2)]
                q8_a = [keep.tile([P, NQ], F8, tag=f"q8{m}", name=f"q8{m}") for m in range(2)]
                qres_a = [keep.tile([P, NQ], F8, tag=f"qr{m}", name=f"qr{m}") for m in range(2)]
                ksq_a = [keep.tile([P, N], F16, tag=f"ks{m}", name=f"ks{m}") for m in range(2)]
                qsq_a = [keep.tile([P, NQ], F16, tag=f"qs{m}", name=f"qs{m}") for m in range(2)]
                q2row = keep.tile([H, NQ], F16, tag="q2r", name="q2r")
                c0_a = keep.tile([H, NQ], F8, tag="c0", name="c0")
                c1t = keep.tile([H, NQ], F16, tag="c1t", name="c1t")
                c1f_a = keep.tile([H, NQ], F8, tag="c1f", name="c1f")
                c2f_a = keep.tile([H, NQ], F8, tag="c2f", name="c2f")
                q2c = keep.tile([P, 64], F16, tag="q2c", name="q2c")
                bias_all = keep.tile([P, P], F32, tag="ball", name="ball")

                # constant rows (B half): p 46:49 ones, p 49 zero.  Engine
                # writes must start at legal partition bases, so stage at
                # partition 0 and DMA into place.
                konst1 = keep.tile([3, N], F8, tag="konst1", name="konst1")
                konst0 = keep.tile([1, N], F8, tag="konst0", name="konst0")
                nc.vector.memset(konst1[:, :], 1.0)
                nc.vector.memset(konst0[:, :], 0.0)
                for i in range(2):
                    nc.sync.dma_start(out=kt8_t[i][46:49, N:2 * N], in_=konst1[:, :])
                    nc.sync.dma_start(out=kt8_t[i][49:50, N:2 * N], in_=konst0[:, :])
                    nc.sync.dma_start(out=qt8_t[i][49:50, NQ:2 * NQ], in_=konst0[:, 0:NQ])

                for m in range(2):
                    nc.vector.tensor_scalar_mul(m8_a[m][:, :], k_t[m][:, :], -2.0)
                    nc.vector.scalar_tensor_tensor(mres_a[m][:, :], k_t[m][:, :],
                                                   -2.0, m8_a[m][:, :],
                                                   op0=ALU.mult, op1=ALU.subtract)
                    nc.vector.tensor_copy(q8_a[m][:, :], q_t[m][:, :])
                    nc.vector.tensor_sub(qres_a[m][:, :], q_t[m][:, :], q8_a[m][:, :])
                    nc.gpsimd.tensor_mul(ksq_a[m][:, :], k_t[m][:, :], k_t[m][:, :])
                    nc.gpsimd.tensor_mul(qsq_a[m][:, :], q_t[m][:, :], q_t[m][:, :])

                # k2 for all heads/key-tiles -> ACT bias table [128 j, h*16+jt]
                k2ps = pd2.tile([P, NQ], F32, tag="d2", name="d2")
                for m in range(2):
                    for g in range(4):
                        for jt in range(NJT):
                            mm(k2ps[:, (m * 4 + g) * NJT + jt:(m * 4 + g) * NJT + jt + 1],
                               ksq_a[m][:, jt * P:(jt + 1) * P],
                               ones4[:, g:g + 1],
                               start=True, stop=True)
                nc.vector.tensor_scalar(bias_all[:, :], k2ps[:, 0:P],
                                        S4, S4 * DELTA,
                                        op0=ALU.mult, op1=ALU.add)

                # q2 for all heads: 1-col transpose matmuls -> DMA to rows
                # interleaved query subsets: chunk e covers queries
                # {i : i mod 8 == e}, so q2c's flat (partition, chunk) order
                # IS the query order and the row DMA needs no transpose.
                q2ps = pd2.tile([P, NQ], F32, tag="d2", name="d2")
                for m in range(2):
                    qsv = qsq_a[m][:, :].rearrange("d (j e) -> d e j", e=8)
                    for g in range(4):
                        for e in range(8):
                            col = (m * 4 + g) * 8 + e
                            mm(q2ps[:, col:col + 1],
                               qsv[:, e, :],
                               ones4[:, g:g + 1],
                               start=True, stop=True)
                nc.vector.tensor_copy(q2c[:, :], q2ps[:, 0:64])
                for h in range(H):
                    nc.sync.dma_start(
                        out=q2row[h:h + 1, :],
                        in_=q2c[:, h * 8:(h + 1) * 8])
                # 3-term fp8 split of q2
                nc.vector.tensor_copy(c0_a[:, :], q2row[:, :])
                nc.vector.tensor_sub(c1t[:, :], q2row[:, :], c0_a[:, :])
                nc.vector.tensor_copy(c1f_a[:, :], c1t[:, :])
                nc.vector.tensor_sub(c2f_a[:, :], c1t[:, :], c1f_a[:, :])

                tail = {}

                def emit_tail(ph, last=False):
                    # deferred normalization of head ph.  The (1,1024) row of
                    # row-sums is reciprocal'd as (128,8) via a scatter DMA
                    # there and back (single-partition recip is ~100x slower).
                    # The 32-row broadcast of the reciprocal row is done by a
                    # log-doubling DMA chain (off the PE); the last head keeps
                    # the PE outer-product since its tail is latency-critical.
                    pmt, pmo, psrc = tail.pop(ph)
                    rs128 = work.tile([P, NQ // P], F32, tag="rs", name="rs")
                    nc.sync.dma_start(out=rs128[:, :], in_=psrc[D:D + 1, :])
                    rr128 = work.tile([P, NQ // P], F16, tag="rr", name="rr")
                    with nc.allow_low_precision(reason="fp16 softmax recip"):
                        nc.vector.reciprocal(rr128[:, :], rs128[:, :])
                    rrow = work.tile([1, NQ], F16, tag="rrow", name="rrow")
                    nc.sync.dma_start(out=rrow[:, :], in_=rr128[:, :])
                    if last:
                        prep = pd2.tile([P, NQ], F32, tag="d2", name="d2")
                        for n in range(NQ // 512):
                            mm(prep[0:D, n * 512:(n + 1) * 512],
                               ones16[0:1, 0:D],
                               rrow[:, n * 512:(n + 1) * 512],
                               start=True, stop=True)
                        nc.vector.tensor_mul(y_t[pmt][pmo:pmo + D, :],
                                             psrc[0:D, :], prep[0:D, :])
                    else:
                        preps = work.tile([D, NQ], F16, tag="preps", name="preps")
                        nc.sync.dma_start(out=preps[0:1, :], in_=rrow[:, :])
                        wseg = 1
                        while wseg < D:
                            nc.sync.dma_start(out=preps[wseg:2 * wseg, :],
                                              in_=preps[0:wseg, :])
                            wseg *= 2
                        nc.vector.tensor_mul(y_t[pmt][pmo:pmo + D, :],
                                             psrc[0:D, :], preps[:, :])

                from contextlib import nullcontext

                for h in range(H):
                    mt, mo = h // 4, (h % 4) * D
                    kt8 = kt8_t[h % 2]
                    qt8 = qt8_t[h % 2]
                    eq = eq_t[h % 2]
                    prio = tc.high_priority(10000) if h == 0 else nullcontext()
                    prio.__enter__()

                    # --- gather this head's DoubleRow operands (DMA) ---
                    # A half = cols 0:N / 0:NQ, B half = cols N:2N / NQ:2NQ
                    nc.sync.dma_start(out=kt8[0:32, 0:N], in_=m8_a[mt][mo:mo + 32, :])
                    nc.sync.dma_start(out=kt8[32:50, 0:N], in_=m8_a[mt][mo:mo + 18, :])
                    nc.sync.dma_start(out=kt8[0:14, N:2 * N], in_=m8_a[mt][mo + 18:mo + 32, :])
                    nc.sync.dma_start(out=kt8[14:46, N:2 * N], in_=mres_a[mt][mo:mo + 32, :])
                    nc.sync.dma_start(out=qt8[0:32, 0:NQ], in_=q8_a[mt][mo:mo + 32, :])
                    nc.sync.dma_start(out=qt8[32:50, 0:NQ], in_=qres_a[mt][mo:mo + 18, :])
                    nc.sync.dma_start(out=qt8[0:14, NQ:2 * NQ], in_=qres_a[mt][mo + 18:mo + 32, :])
                    nc.sync.dma_start(out=qt8[14:46, NQ:2 * NQ], in_=q8_a[mt][mo:mo + 32, :])
                    nc.sync.dma_start(out=qt8[46:47, NQ:2 * NQ], in_=c0_a[h:h + 1, :])
                    nc.sync.dma_start(out=qt8[47:48, NQ:2 * NQ], in_=c1f_a[h:h + 1, :])
                    nc.sync.dma_start(out=qt8[48:49, NQ:2 * NQ], in_=c2f_a[h:h + 1, :])

                    kt8v = kt8[:, :].rearrange("p (two c) -> p two c", two=2)
                    qt8v = qt8[:, :].rearrange("p (two c) -> p two c", two=2)

                    # --- dist2' (fp8 DoubleRow) -> w = (c/4)*dist (ACT) ---
                    for jt in range(NJT):
                        psd = pd2.tile([P, NQ], F32, tag="d2", name="d2")
                        for n in range(NQ // 512):
                            nc.tensor.matmul(
                                psd[:, n * 512:(n + 1) * 512],
                                kt8v[:, :, jt * P:(jt + 1) * P],
                                qt8v[:, :, n * 512:(n + 1) * 512],
                                start=True, stop=True, perf_mode=DR)
                        nc.scalar.activation(
                            eq[:, jt * NQ:(jt + 1) * NQ],
                            psd[:, :], AF.Sqrt,
                            bias=bias_all[:, h * NJT + jt:h * NJT + jt + 1],
                            scale=S4)

                    prio.__exit__(None, None, None)

                    # head h-1 tail: PE ops queue here, after dist2(h)
                    if h - 1 in tail:
                        emit_tail(h - 1)

                    # --- E = p(w)^4 = exp(-scale*dist): one custom-DVE pass ---
                    for cch in range(4):
                        seg = eq[:, cch * 4 * NQ:(cch + 1) * 4 * NQ]
                        nc.vector._custom_dve(exp_op, out=seg, in0=seg,
                                              in1=a0_t[:, 0:1],
                                              s0=A3, s1=A2, imm2=A1)

                    # --- attn @ v with fused row-sums (ones col of vt) ---
                    pso = po.tile([VTW, NQ], F32, tag="o", name="o")
                    for jt in range(NJT):
                        for n in range(NQ // 512):
                            mm(pso[:, n * 512:(n + 1) * 512],
                               vt_big[:, jt * VSTRIDE + h * VTW:
                                      jt * VSTRIDE + (h + 1) * VTW],
                               eq[:, jt * NQ + n * 512:jt * NQ + (n + 1) * 512],
                               start=(jt == 0), stop=(jt == NJT - 1))
                    # SBUF staging: DMA cannot read PSUM, and tensor_mul
                    # cannot take two PSUM operands; also frees the po slot.
                    psrc = po_s[h % 2]
                    nc.vector.tensor_copy(psrc[:, :], pso[:, :])
                    tail[h] = (mt, mo, psrc)

                # last head's tail
                emit_tail(H - 1, last=True)

            # ======== Phase C: output projection + bias (bias via ACT) ========
            # k=0 accumulations (need only heads 0-3, i.e. y_t[0]) are all
            # emitted first so they overlap the last head's tail latency.
            with tc.tile_pool(name="pz", bufs=1, space="PSUM") as pz:
                z_r = z_d[:].rearrange("(t p) n -> t p n", p=P)
                pzt = []
                for m in range(KT):
                    ps = pz.tile([P, NQ], F32, tag=f"z{m}", name=f"z{m}")
                    pzt.append(ps)
                    for n in range(NQ // 512):
                        mm(ps[:, n * 512:(n + 1) * 512],
                           wo_t[0][:, m * P:(m + 1) * P],
                           y_t[0][:, n * 512:(n + 1) * 512],
                           start=True, stop=False)
                for m in range(KT):
                    ps = pzt[m]
                    for n in range(NQ // 512):
                        mm(ps[:, n * 512:(n + 1) * 512],
                           wo_t[1][:, m * P:(m + 1) * P],
                           y_t[1][:, n * 512:(n + 1) * 512],
                           start=False, stop=True)
                    for n in range(NQ // 512):
                        zt = work.tile([P, 512], F32, tag="ytmp", name="ytmp")
                        nc.scalar.activation(zt[:, :],
                                             ps[:, n * 512:(n + 1) * 512],
                                             AF.Identity,
                                             bias=b_t[:, m:m + 1], scale=1.0)
                        nc.sync.dma_start(out=z_r[m][:, n * 512:(n + 1) * 512],
                                          in_=zt[:, :])

    nc.compile()
    return nc


def make_in_maps(x, w_qkv, w_out, b_out):
    x = np.asarray(x, dtype=np.float32)
    w_qkv = np.asarray(w_qkv, dtype=np.float32)
    w_out = np.asarray(w_out, dtype=np.float32)
    b_out = np.asarray(b_out, dtype=np.float32)
    w_qT = np.ascontiguousarray(w_qkv[0:INNER, :].T).astype(np.float16)
    w_kvT = np.ascontiguousarray(w_qkv[INNER:3 * INNER, :].T).astype(np.float16)
    w_oT = np.ascontiguousarray(w_out.T).astype(np.float16)
    x16 = x.astype(np.float16)
    in_maps = []
    for c in range(8):
        b, half = c // 2, c % 2
        in_maps.append({
            "xq": np.ascontiguousarray(x16[b][:, half * NQ:(half + 1) * NQ]),
            "xkv": np.ascontiguousarray(x16[b]),
            "wq": w_qT,
            "wkv": w_kvT,
            "wo": w_oT,
            "b": b_out,
        })
    return in_maps


def assemble_output(results):
    out = np.empty((B, DIM, N), dtype=np.float32)
    for c in range(8):
        b, half = c // 2, c % 2
        out[b][:, half * NQ:(half + 1) * NQ] = results[c]["z"]
    return out


_prog_cache = {}


def kernel(x, w_qkv, w_out, b_out):
    from concourse.bass_utils import run_bass_kernel_spmd
    if "nc" not in _prog_cache:
        _prog_cache["nc"] = build_program()
    nc = _prog_cache["nc"]
    in_maps = make_in_maps(x, w_qkv, w_out, b_out)
    res = run_bass_kernel_spmd(nc, in_maps, list(range(8)))
    return assemble_output(res.results)


# revision 41
# speedup vs baseline: 1.0246x; 1.0246x over previous
"""L2-distance attention (B=4, DIM=512, N=2048, H=8, D=32) on 8 trn2 NeuronCores.

Sharding: core c handles batch b = c//2, query-half = c%2 (1024 queries, all
2048 keys, all 8 heads).  Output is a pure concat - no cross-core reduce.

Design (vs the 382us fp32r baseline; measured ~326-333us):
  * fp16 inputs/projections (host-side cast halves DMA; PE still 1 cyc/col).
  * dist2 via fp8e4m3 *DoubleRow* matmuls (0.5 cyc/col) with full error
    compensation: DoubleRow contraction rows are free, so the q/k fp8
    rounding residuals ride extra row-groups (m8,q8)+(m8,qres)+(mres,q8)
    and a 3-term fp8 split of the q2 row - end-to-end error stays at the
    fp16 level (8.4e-4).  k2[j] is per-PSUM-partition, so it rides the
    ACT sqrt *bias port* exactly (computed by 1-column transpose matmuls).
    Per-head DoubleRow operand tiles are DMA-gathered from whole-tensor
    fp8 staging tiles; the tensor engine never pays for the layout.
  * ACT does ONLY the sqrt pass (w = (c/4)*sqrt(d2+delta), c=scale*log2 e),
    one table load total.  The exp pass moved off ACT entirely:
  * exp via ONE custom-DVE instruction per chunk (registered at build
    time): a minimax cubic p(w) of 2^-w on [0, 0.95], squared twice in
    the 8-stage DVE ALU pipeline: E = p(w)^4 = 2^(-4w) = exp(-scale*dist),
    rel err 2.4e-4, fp16 in/out, in-place.
  * PSUM in 8 banks: pd2 2x[128,1024]f32 + po 2x[33,1024]f32; the
    k2/q2/prep needs borrow dist2 pool slots.
  * Remaining wall-clock is set by a hardware PE duty-cycle governor
    (3.4us at 8/8 then 30.7us at 4/8 under sustained matmul load) and by
    the cost of fp8<->fp16 weight-mode switches (~0.4-0.7us each), which
    rules out finer dist2/attnv interleaving (measured: 4-jt groups
    +12us, 2-mm alternation +80us, half-jt groups +22us).
"""

import numpy as np

import concourse.bass as bass
import concourse.mybir as mybir
import concourse.tile as tile
from concourse import bacc

F32 = mybir.dt.float32
F16 = mybir.dt.float16
F8 = mybir.dt.float8e4
AF = mybir.ActivationFunctionType
ALU = mybir.AluOpType
DR = mybir.MatmulPerfMode.DoubleRow

B, DIM, N = 4, 512, 2048
H, D = 8, 32
INNER = H * D            # 256
NQ = N // 2              # 1024 queries per core
P = 128
KT = DIM // P            # 4 contraction tiles for the projections
NJT = N // P             # 16 key tiles
VTW = D + 1              # 33: v columns + ones column per head
VSTRIDE = H * VTW        # 264 columns per key-tile block of vt
SCALE = float(D) ** -0.5
DELTA = 0.05             # dist2 guard against fp16 rounding (sqrt>0)
C_EXP = SCALE * float(np.log2(np.e))   # u = C_EXP * dist; E = 2^-u
S4 = (C_EXP * C_EXP) / 16.0            # sqrt scale so ACT emits w = u/4

# minimax cubic for 2^-w on [0, 0.95] (rel err 5.9e-5; E=p^4 -> 2.4e-4)
A3, A2, A1, A0 = -0.0398455, 0.23114166, -0.69136122, 0.99994121


def _register_exp_op():
    """Register the quartic-exp custom DVE op (idempotent).

    body: sq(sq(((a3*w + a2)*w + a1)*w + a0)) = p(w)^4 ~= 2^(-4w).
    a3,a2 ride C0,C1; a1 rides imm2 (compile-time literal); a0 rides the
    spilled C3 (delivered via in1 as a [P,1] broadcast).
    """
    from concourse import dve_ops as dops
    from concourse.dve_spec import (
        C0, C1, C2, C3, Spec, Src0, _has_src1, _spill_c3_to_src1, lower, sq,
    )
    from concourse.dve_uop import DveOpSpec

    NAME = "EXPQ4_ANT"
    for op in dops.OPS:
        if op.name == NAME:
            return op

    w = Src0
    p = ((C0 * w + C1) * w + C2) * w + C3
    body = _spill_c3_to_src1(sq(sq(p)))
    spec = Spec(
        body=body,
        reference=lambda in0, in1, s0, s1, imm2: (
            (((s0 * in0 + s1) * in0 + imm2) * in0 + in1) ** 4
        ),
    )
    row = dops._CUSTOM_DVE_ROW_BASE + len(dops.OPS)
    shas = {}
    for ver in ("v3", "v4"):
        tmp = DveOpSpec(name=NAME, opcode=row, uops=lower(spec, ver=ver),
                        rd1_en=_has_src1(spec))
        shas[ver] = tmp.sha(ver)
    op = dops.DveOp(NAME, spec, subdim=False, uops_sha=shas)
    dops.OPS.append(op)
    dops._SUB_OPCODE_FOR_NAME[NAME] = row
    dops.CUSTOM_DVE_SPECS[NAME] = spec
    return op


def make_mm(nc):
    def mm(out, lhsT, rhs, start, stop):
        nc.tensor.matmul(out, lhsT, rhs, start=start, stop=stop)
    return mm


def build_program() -> bass.Bass:
    exp_op = _register_exp_op()
    nc = bacc.Bacc("TRN2", target_bir_lowering=False, debug=False)

    xq_d = nc.declare_dram_parameter("xq", [DIM, NQ], F16, isOutput=False)
    xkv_d = nc.declare_dram_parameter("xkv", [DIM, N], F16, isOutput=False)
    wq_d = nc.declare_dram_parameter("wq", [DIM, INNER], F16, isOutput=False)
    wkv_d = nc.declare_dram_parameter("wkv", [DIM, 2 * INNER], F16, isOutput=False)
    wo_d = nc.declare_dram_parameter("wo", [INNER, DIM], F16, isOutput=False)
    b_d = nc.declare_dram_parameter("b", [DIM], F32, isOutput=False)
    z_d = nc.declare_dram_parameter("z", [DIM, NQ], F32, isOutput=True)

    with tile.TileContext(nc) as tc:
        mm = make_mm(nc)
        with tc.tile_pool(name="keep", bufs=1) as keep, \
             tc.tile_pool(name="work", bufs=2) as work:

            # ---- persistent tiles ----
            q_t = [keep.tile([P, NQ], F16, tag=f"q{m}", name=f"q{m}") for m in range(2)]
            k_t = [keep.tile([P, N], F16, tag=f"k{m}", name=f"k{m}") for m in range(2)]
            vt_big = keep.tile([P, NJT * VSTRIDE], F16, tag="vt", name="vt")
            y_t = [keep.tile([P, NQ], F16, tag=f"y{m}", name=f"y{m}") for m in range(2)]
            wo_t = [keep.tile([P, DIM], F16, tag=f"wo{m}", name=f"wo{m}") for m in range(2)]
            b_t = keep.tile([P, KT], F32, tag="bias", name="bias")
            ones16 = keep.tile([64, 32], F16, tag="ones16", name="ones16")
            onesP16 = keep.tile([P, 1], F16, tag="onesP16", name="onesP16")
            a0_t = keep.tile([P, 1], F32, tag="a0c", name="a0c")

            nc.vector.memset(ones16[:, :], 1.0)
            nc.vector.memset(onesP16[:, :], 1.0)
            # masked ones: col g = 1.0 on partitions [32g, 32g+32), else 0
            ones4 = keep.tile([P, 4], F16, tag="ones4", name="ones4")
            nc.vector.memset(ones4[:, :], 0.0)
            for g in range(4):
                nc.vector.memset(ones4[g * 32:(g + 1) * 32, g:g + 1], 1.0)
            nc.vector.memset(a0_t[:, :], A0)

            # ======== Phase A: projections ========
            with tc.tile_pool(name="xw", bufs=1) as xw, \
                 tc.tile_pool(name="pp", bufs=3, space="PSUM") as pp:
                wq_t = [xw.tile([P, INNER], F16, tag=f"wq{k}", name=f"wq{k}") for k in range(KT)]
                wkv_t = [xw.tile([P, 2 * INNER], F16, tag=f"wkv{k}", name=f"wkv{k}") for k in range(KT)]
                xq_t = [xw.tile([P, NQ], F16, tag=f"xq{k}", name=f"xq{k}") for k in range(KT)]
                xkv_t = [xw.tile([P, N], F16, tag=f"xkv{k}", name=f"xkv{k}") for k in range(KT)]

                xq_r = xq_d[:].rearrange("(t p) n -> t p n", p=P)
                xkv_r = xkv_d[:].rearrange("(t p) n -> t p n", p=P)
                wq_r = wq_d[:].rearrange("(t p) o -> t p o", p=P)
                wkv_r = wkv_d[:].rearrange("(t p) o -> t p o", p=P)
                for k in range(KT):
                    nc.sync.dma_start(out=xq_t[k][:, :], in_=xq_r[k])
                    nc.sync.dma_start(out=xkv_t[k][:, :], in_=xkv_r[k])
                    nc.sync.dma_start(out=wq_t[k][:, :], in_=wq_r[k])
                    nc.sync.dma_start(out=wkv_t[k][:, :], in_=wkv_r[k])
                wo_r = wo_d[:].rearrange("(t p) o -> t p o", p=P)
                for m in range(2):
                    nc.sync.dma_start(out=wo_t[m][:, :], in_=wo_r[m])
                nc.sync.dma_start(out=b_t[:, :], in_=b_d[:].rearrange("(t p) -> p t", p=P))

                # q projection: (DIM x NQ) -> (INNER x NQ)
                for m in range(2):
                    for n in range(NQ // 512):
                        ps = pp.tile([P, 512], F32, tag="proj", name="proj")
                        for k in range(KT):
                            mm(ps[:, :],
                               wq_t[k][:, m * P:(m + 1) * P],
                               xq_t[k][:, n * 512:(n + 1) * 512],
                               start=(k == 0), stop=(k == KT - 1))
                        nc.vector.tensor_copy(q_t[m][:, n * 512:(n + 1) * 512], ps[:, :])

                # k projection: (DIM x N) -> (INNER x N)   (wkv cols 0:256)
                # PSUM->SBUF copies go through ACT (identity, no table cost);
                # DVE is the tighter engine.
                for m in range(2):
                    for n in range(N // 512):
                        ps = pp.tile([P, 512], F32, tag="proj", name="proj")
                        for k in range(KT):
                            mm(ps[:, :],
                               wkv_t[k][:, m * P:(m + 1) * P],
                               xkv_t[k][:, n * 512:(n + 1) * 512],
                               start=(k == 0), stop=(k == KT - 1))
                        nc.scalar.activation(k_t[m][:, n * 512:(n + 1) * 512],
                                             ps[:, :], AF.Identity, scale=1.0)

                # v^T projection: per key tile jt, (128 j x 256 d), strided into
                # vt_big so each head's 32 columns sit next to its ones column.
                nc.vector.tensor_copy(
                    vt_big[:, :].rearrange("p (a c) -> p a c", c=VTW)[:, :, D:D + 1],
                    onesP16[:, 0:1].to_broadcast((P, P, 1)))
                for jt in range(NJT):
                    ps = pp.tile([P, INNER], F32, tag="vtps", name="vtps")
                    for k in range(KT):
                        mm(ps[:, :],
                           xkv_t[k][:, jt * P:(jt + 1) * P],
                           wkv_t[k][:, INNER:2 * INNER],
                           start=(k == 0), stop=(k == KT - 1))
                    dst = vt_big[:, jt * VSTRIDE:(jt + 1) * VSTRIDE] \
                        .rearrange("p (h c) -> p h c", c=VTW)[:, :, 0:D]
                    src = ps[:, :].rearrange("p (h d) -> p h d", d=D)
                    nc.scalar.activation(dst, src, AF.Identity, scale=1.0)

            # ======== Phase B: attention heads ========
            # dist2 via fp8e4m3 DoubleRow matmuls (0.5 cyc/col) with full
            # error compensation: extra contraction rows are free, so the
            # logical 100-row contraction is
            #   rows  0:32  (m8[d],  q8[d])     m8 = fp8(-2k), q8 = fp8(q)
            #   rows 32:64  (m8[d],  qres[d])   qres = fp8(q - q8)
            #   rows 64:96  (mres[d],q8[d])     mres = fp8(-2k - m8)
            #   rows 96:99  (1, c0/c1f/c2f)     3-term fp8 split of q2
            #   row  99     (0, 0)
            # mapped to 50 DoubleRow partition-pairs: r = half*50 + p.
            # k2[j] stays exact on the ACT sqrt bias port (per-partition).
            with tc.tile_pool(name="pd2", bufs=2, space="PSUM") as pd2, \
                 tc.tile_pool(name="po", bufs=2, space="PSUM") as po:
                KP = 50
                kt8_t = [keep.tile([KP, 2 * N], F8, tag=f"kt8{i}", name=f"kt8{i}")
                         for i in range(2)]
                qt8_t = [keep.tile([KP, 2 * NQ], F8, tag=f"qt8{i}", name=f"qt8{i}")
                         for i in range(2)]
                eq_t = [keep.tile([P, NJT * NQ], F16, tag=f"eq{i}", name=f"eq{i}")
                        for i in range(2)]
                po_s = [keep.tile([VTW, NQ], F32, tag=f"pos{i}", name=f"pos{i}")
                        for i in range(2)]
                # fp8 staging (whole-tensor ops; per-head slices DMA-gathered)
                m8_a = [keep.tile([P, N], F8, tag=f"m8{m}", name=f"m8{m}") for m in range(2)]
                mres_a = [keep.tile([P, N], F8, tag=f"mr{m}", name=f"mr{m}") for m in range(2)]
                q8_a = [keep.tile([P, NQ], F8, tag=f"q8{m}", name=f"q8{m}") for m in range(2)]
                qres_a = [keep.tile([P, NQ], F8, tag=f"qr{m}", name=f"qr{m}") for m in range(2)]
                ksq_a = [keep.tile([P, N], F16, tag=f"ks{m}", name=f"ks{m}") for m in range(2)]
                qsq_a = [keep.tile([P, NQ], F16, tag=f"qs{m}", name=f"qs{m}") for m in range(2)]
                q2row = keep.tile([H, NQ], F16, tag="q2r", name="q2r")
                c0_a = keep.tile([H, NQ], F8, tag="c0", name="c0")
                c1t = keep.tile([H, NQ], F16, tag="c1t", name="c1t")
                c1f_a = keep.tile([H, NQ], F8, tag="c1f", name="c1f")
                c2f_a = keep.tile([H, NQ], F8, tag="c2f", name="c2f")
                q2c = keep.tile([P, 64], F16, tag="q2c", name="q2c")
                bias_all = keep.tile([P, P], F32, tag="ball", name="ball")

                # constant rows (B half): p 46:49 ones, p 49 zero.  Engine
                # writes must start at legal partition bases, so stage at
                # partition 0 and DMA into place.
                konst1 = keep.tile([3, N], F8, tag="konst1", name="konst1")
                konst0 = keep.tile([1, N], F8, tag="konst0", name="konst0")
                nc.vector.memset(konst1[:, :], 1.0)
                nc.vector.memset(konst0[:, :], 0.0)
                for i in range(2):
                    nc.sync.dma_start(out=kt8_t[i][46:49, N:2 * N], in_=konst1[:, :])
                    nc.sync.dma_start(out=kt8_t[i][49:50, N:2 * N], in_=konst0[:, :])
                    nc.sync.dma_start(out=qt8_t[i][49:50, NQ:2 * NQ], in_=konst0[:, 0:NQ])

                for m in range(2):
                    nc.vector.tensor_scalar_mul(m8_a[m][:, :], k_t[m][:, :], -2.0)
                    nc.vector.scalar_tensor_tensor(mres_a[m][:, :], k_t[m][:, :],
                                                   -2.0, m8_a[m][:, :],
                                                   op0=ALU.mult, op1=ALU.subtract)
                    nc.vector.tensor_copy(q8_a[m][:, :], q_t[m][:, :])
                    nc.vector.tensor_sub(qres_a[m][:, :], q_t[m][:, :], q8_a[m][:, :])
                    nc.gpsimd.tensor_mul(ksq_a[m][:, :], k_t[m][:, :], k_t[m][:, :])
                    nc.gpsimd.tensor_mul(qsq_a[m][:, :], q_t[m][:, :], q_t[m][:, :])

                # k2 for all heads/key-tiles -> ACT bias table [128 j, h*16+jt]
                k2ps = pd2.tile([P, NQ], F32, tag="d2", name="d2")
                for m in range(2):
                    for g in range(4):
                        for jt in range(NJT):
                            mm(k2ps[:, (m * 4 + g) * NJT + jt:(m * 4 + g) * NJT + jt + 1],
                               ksq_a[m][:, jt * P:(jt + 1) * P],
                               ones4[:, g:g + 1],
                               start=True, stop=True)
                nc.vector.tensor_scalar(bias_all[:, :], k2ps[:, 0:P],
                                        S4, S4 * DELTA,
                                        op0=ALU.mult, op1=ALU.add)

                # q2 for all heads: 1-col transpose matmuls -> DMA to rows
                # interleaved query subsets: chunk e covers queries
                # {i : i mod 8 == e}, so q2c's flat (partition, chunk) order
                # IS the query order and the row DMA needs no transpose.
                q2ps = pd2.tile([P, NQ], F32, tag="d2", name="d2")
                for m in range(2):
                    qsv = qsq_a[m][:, :].rearrange("d (j e) -> d e j", e=8)
                    for g in range(4):
                        for e in range(8):
                            col = (m * 4 + g) * 8 + e
                            mm(q2ps[:, col:col + 1],
                               qsv[:, e, :],
                               ones4[:, g:g + 1],
                               start=True, stop=True)
                nc.vector.tensor_copy(q2c[:, :], q2ps[:, 0:64])
                for h in range(H):
                    nc.sync.dma_start(
                        out=q2row[h:h + 1, :],
                        in_=q2c[:, h * 8:(h + 1) * 8])
                # 3-term fp8 split of q2
                nc.vector.tensor_copy(c0_a[:, :], q2row[:, :])
                nc.vector.tensor_sub(c1t[:, :], q2row[:, :], c0_a[:, :])
                nc.vector.tensor_copy(c1f_a[:, :], c1t[:, :])
                nc.vector.tensor_sub(c2f_a[:, :], c1t[:, :], c1f_a[:, :])

                tail = {}

                def emit_tail(ph, last=False):
                    # deferred normalization of head ph.  The (1,1024) row of
                    # row-sums is reciprocal'd as (128,8) via a scatter DMA
                    # there and back (single-partition recip is ~100x slower).
                    # The 32-row broadcast of the reciprocal row is done by a
                    # log-doubling DMA chain (off the PE); the last head keeps
                    # the PE outer-product since its tail is latency-critical.
                    pmt, pmo, psrc = tail.pop(ph)
                    rs128 = work.tile([P, NQ // P], F32, tag="rs", name="rs")
                    nc.sync.dma_start(out=rs128[:, :], in_=psrc[D:D + 1, :])
                    rr128 = work.tile([P, NQ // P], F16, tag="rr", name="rr")
                    with nc.allow_low_precision(reason="fp16 softmax recip"):
                        nc.vector.reciprocal(rr128[:, :], rs128[:, :])
                    rrow = work.tile([1, NQ], F16, tag="rrow", name="rrow")
                    nc.sync.dma_start(out=rrow[:, :], in_=rr128[:, :])
                    if last:
                        prep = pd2.tile([P, NQ], F32, tag="d2", name="d2")
                        for n in range(NQ // 512):
                            mm(prep[0:D, n * 512:(n + 1) * 512],
                               ones16[0:1, 0:D],
                               rrow[:, n * 512:(n + 1) * 512],
                               start=True, stop=True)
                        nc.vector.tensor_mul(y_t[pmt][pmo:pmo + D, :],
                                             psrc[0:D, :], prep[0:D, :])
                    else:
                        preps = work.tile([D, NQ], F16, tag="preps", name="preps")
                        nc.sync.dma_start(out=preps[0:1, :], in_=rrow[:, :])
                        wseg = 1
                        while wseg < D:
                            nc.sync.dma_start(out=preps[wseg:2 * wseg, :],
                                              in_=preps[0:wseg, :])
                            wseg *= 2
                        nc.vector.tensor_mul(y_t[pmt][pmo:pmo + D, :],
                                             psrc[0:D, :], preps[:, :])

                from contextlib import nullcontext

                for h in range(H):
                    mt, mo = h // 4, (h % 4) * D
                    kt8 = kt8_t[h % 2]
                    qt8 = qt8_t[h % 2]
                    eq = eq_t[h % 2]
                    prio = tc.high_priority(10000) if h == 0 else nullcontext()
                    prio.__enter__()

                    # --- gather this head's DoubleRow operands (DMA) ---
                    # A half = cols 0:N / 0:NQ, B half = cols N:2N / NQ:2NQ
                    nc.sync.dma_start(out=kt8[0:32, 0:N], in_=m8_a[mt][mo:mo + 32, :])
                    nc.sync.dma_start(out=kt8[32:50, 0:N], in_=m8_a[mt][mo:mo + 18, :])
                    nc.sync.dma_start(out=kt8[0:14, N:2 * N], in_=m8_a[mt][mo + 18:mo + 32, :])
                    nc.sync.dma_start(out=kt8[14:46, N:2 * N], in_=mres_a[mt][mo:mo + 32, :])
                    nc.sync.dma_start(out=qt8[0:32, 0:NQ], in_=q8_a[mt][mo:mo + 32, :])
                    nc.sync.dma_start(out=qt8[32:50, 0:NQ], in_=qres_a[mt][mo:mo + 18, :])
                    nc.sync.dma_start(out=qt8[0:14, NQ:2 * NQ], in_=qres_a[mt][mo + 18:mo + 32, :])
                    nc.sync.dma_start(out=qt8[14:46, NQ:2 * NQ], in_=q8_a[mt][mo:mo + 32, :])
                    nc.sync.dma_start(out=qt8[46:47, NQ:2 * NQ], in_=c0_a[h:h + 1, :])
                    nc.sync.dma_start(out=qt8[47:48, NQ:2 * NQ], in_=c1f_a[h:h + 1, :])
                    nc.sync.dma_start(out=qt8[48:49, NQ:2 * NQ], in_=c2f_a[h:h + 1, :])

                    kt8v = kt8[:, :].rearrange("p (two c) -> p two c", two=2)
                    qt8v = qt8[:, :].rearrange("p (two c) -> p two c", two=2)

                    # --- dist2' (fp8 DoubleRow) -> w = (c/4)*dist (ACT) ---
                    for jt in range(NJT):
                        psd = pd2.tile([P, NQ], F32, tag="d2", name="d2")
                        for n in range(NQ // 512):
                            nc.tensor.matmul(
                                psd[:, n * 512:(n + 1) * 512],
                                kt8v[:, :, jt * P:(jt + 1) * P],
                                qt8v[:, :, n * 512:(n + 1) * 512],
                                start=True, stop=True, perf_mode=DR)
                        nc.scalar.activation(
                            eq[:, jt * NQ:(jt + 1) * NQ],
                            psd[:, :], AF.Sqrt,
                            bias=bias_all[:, h * NJT + jt:h * NJT + jt + 1],
                            scale=S4)

                    prio.__exit__(None, None, None)

                    # head h-1 tail: PE ops queue here, after dist2(h)
                    # (heads 6/7 keep the PE broadcast: their tails gate the
                    # output projection, and the DMA chain adds ~5us latency)
                    if h - 1 in tail:
                        emit_tail(h - 1, last=(h - 1 >= H - 2))

                    # --- E = p(w)^4 = exp(-scale*dist): one custom-DVE pass ---
                    for cch in range(4):
                        seg = eq[:, cch * 4 * NQ:(cch + 1) * 4 * NQ]
                        nc.vector._custom_dve(exp_op, out=seg, in0=seg,
                                              in1=a0_t[:, 0:1],
                                              s0=A3, s1=A2, imm2=A1)

                    # --- attn @ v with fused row-sums (ones col of vt) ---
                    pso = po.tile([VTW, NQ], F32, tag="o", name="o")
                    for jt in range(NJT):
                        for n in range(NQ // 512):
                            mm(pso[:, n * 512:(n + 1) * 512],
                               vt_big[:, jt * VSTRIDE + h * VTW:
                                      jt * VSTRIDE + (h + 1) * VTW],
                               eq[:, jt * NQ + n * 512:jt * NQ + (n + 1) * 512],
                               start=(jt == 0), stop=(jt == NJT - 1))
                    # SBUF staging: DMA cannot read PSUM, and tensor_mul
                    # cannot take two PSUM operands; also frees the po slot.
                    psrc = po_s[h % 2]
                    nc.vector.tensor_copy(psrc[:, :], pso[:, :])
                    tail[h] = (mt, mo, psrc)

                # last head's tail
                emit_tail(H - 1, last=True)

            # ======== Phase C: output projection + bias (bias via ACT) ========
            # k=0 accumulations (need only heads 0-3, i.e. y_t[0]) are all
            # emitted first so they overlap the last head's tail latency.
            with tc.tile_pool(name="pz", bufs=1, space="PSUM") as pz:
                z_r = z_d[:].rearrange("(t p) n -> t p n", p=P)
                pzt = []
                for m in range(KT):
                    ps = pz.tile([P, NQ], F32, tag=f"z{m}", name=f"z{m}")
                    pzt.append(ps)
                    for n in range(NQ // 512):
                        mm(ps[:, n * 512:(n + 1) * 512],
                           wo_t[0][:, m * P:(m + 1) * P],
                           y_t[0][:, n * 512:(n + 1) * 512],
                           start=True, stop=False)
                for m in range(KT):
                    ps = pzt[m]
                    for n in range(NQ // 512):
                        mm(ps[:, n * 512:(n + 1) * 512],
                           wo_t[1][:, m * P:(m + 1) * P],
                           y_t[1][:, n * 512:(n + 1) * 512],
                           start=False, stop=True)
                    for n in range(NQ // 512):
                        zt = work.tile([P, 512], F32, tag="ytmp", name="ytmp")
                        nc.scalar.activation(zt[:, :],
                                             ps[:, n * 512:(n + 1) * 512],
                                             AF.Identity,
                                             bias=b_t[:, m:m + 1], scale=1.0)
                        nc.sync.dma_start(out=z_r[m][:, n * 512:(n + 1) * 512],
                                          in_=zt[:, :])

    nc.compile()
    return nc


def make_in_maps(x, w_qkv, w_out, b_out):
    x = np.asarray(x, dtype=np.float32)
    w_qkv = np.asarray(w_qkv, dtype=np.float32)
    w_out = np.asarray(w_out, dtype=np.float32)
    b_out = np.asarray(b_out, dtype=np.float32)
    w_qT = np.ascontiguousarray(w_qkv[0:INNER, :].T).astype(np.float16)
    w_kvT = np.ascontiguousarray(w_qkv[INNER:3 * INNER, :].T).astype(np.float16)
    w_oT = np.ascontiguousarray(w_out.T).astype(np.float16)
    x16 = x.astype(np.float16)
    in_maps = []
    for c in range(8):
        b, half = c // 2, c % 2
        in_maps.append({
            "xq": np.ascontiguousarray(x16[b][:, half * NQ:(half + 1) * NQ]),
            "xkv": np.ascontiguousarray(x16[b]),
            "wq": w_qT,
            "wkv": w_kvT,
            "wo": w_oT,
            "b": b_out,
        })
    return in_maps


def assemble_output(results):
    out = np.empty((B, DIM, N), dtype=np.float32)
    for c in range(8):
        b, half = c // 2, c % 2
        out[b][:, half * NQ:(half + 1) * NQ] = results[c]["z"]
    return out


_prog_cache = {}


def kernel(x, w_qkv, w_out, b_out):
    from concourse.bass_utils import run_bass_kernel_spmd
    if "nc" not in _prog_cache:
        _prog_cache["nc"] = build_program()
    nc = _prog_cache["nc"]
    in_maps = make_in_maps(x, w_qkv, w_out, b_out)
    res = run_bass_kernel_spmd(nc, in_maps, list(range(8)))
    return assemble_output(res.results)


# revision 44
# speedup vs baseline: 1.1570x; 1.1291x over previous
"""L2-distance attention (B=4, DIM=512, N=2048, H=8, D=32) on 8 trn2 NeuronCores.

Sharding: core c handles batch b = c//2, query-half = c%2 (1024 queries, all
2048 keys, all 8 heads).  Output is a pure concat - no cross-core reduce.

Design (vs the 382us fp32r baseline; measured ~326-333us):
  * fp16 inputs/projections (host-side cast halves DMA; PE still 1 cyc/col).
  * dist2 via fp8e4m3 *DoubleRow* matmuls (0.5 cyc/col) with full error
    compensation: DoubleRow contraction rows are free, so the q/k fp8
    rounding residuals ride extra row-groups (m8,q8)+(m8,qres)+(mres,q8)
    and a 3-term fp8 split of the q2 row - end-to-end error stays at the
    fp16 level (8.4e-4).  k2[j] is per-PSUM-partition, so it rides the
    ACT sqrt *bias port* exactly (computed by 1-column transpose matmuls).
    Per-head DoubleRow operand tiles are DMA-gathered from whole-tensor
    fp8 staging tiles; the tensor engine never pays for the layout.
  * ACT does ONLY the sqrt pass (w = (c/4)*sqrt(d2+delta), c=scale*log2 e),
    one table load total.  The exp pass moved off ACT entirely:
  * exp via ONE custom-DVE instruction per chunk (registered at build
    time): a minimax cubic p(w) of 2^-w on [0, 0.95], squared twice in
    the 8-stage DVE ALU pipeline: E = p(w)^4 = 2^(-4w) = exp(-scale*dist),
    rel err 2.4e-4, fp16 in/out, in-place.
  * PSUM in 8 banks: pd2 2x[128,1024]f32 + po 2x[33,1024]f32; the
    k2/q2/prep needs borrow dist2 pool slots.
  * Remaining wall-clock is set by a hardware PE duty-cycle governor
    (3.4us at 8/8 then 30.7us at 4/8 under sustained matmul load) and by
    the cost of fp8<->fp16 weight-mode switches (~0.4-0.7us each), which
    rules out finer dist2/attnv interleaving (measured: 4-jt groups
    +12us, 2-mm alternation +80us, half-jt groups +22us).
"""

import numpy as np

import concourse.bass as bass
import concourse.mybir as mybir
import concourse.tile as tile
from concourse import bacc

F32 = mybir.dt.float32
F16 = mybir.dt.float16
F8 = mybir.dt.float8e4
AF = mybir.ActivationFunctionType
ALU = mybir.AluOpType
DR = mybir.MatmulPerfMode.DoubleRow

B, DIM, N = 4, 512, 2048
H, D = 8, 32
INNER = H * D            # 256
NQ = N // 2              # 1024 queries per core
P = 128
KT = DIM // P            # 4 contraction tiles for the projections
NJT = N // P             # 16 key tiles
VTW = D + 1              # 33: v columns + ones column per head
VSTRIDE = H * VTW        # 264 columns per key-tile block of vt
SCALE = float(D) ** -0.5
DELTA = 0.05             # dist2 guard against fp16 rounding (sqrt>0)
C_EXP = SCALE * float(np.log2(np.e))   # u = C_EXP * dist; E = 2^-u
S4 = (C_EXP * C_EXP) / 16.0            # sqrt scale so ACT emits w = u/4

# minimax cubic for 2^-w on [0, 0.95] (rel err 5.9e-5; E=p^4 -> 2.4e-4)
A3, A2, A1, A0 = -0.0398455, 0.23114166, -0.69136122, 0.99994121


def _register_exp_op():
    """Register the quartic-exp custom DVE op (idempotent).

    body: sq(sq(((a3*w + a2)*w + a1)*w + a0)) = p(w)^4 ~= 2^(-4w).
    a3,a2 ride C0,C1; a1 rides imm2 (compile-time literal); a0 rides the
    spilled C3 (delivered via in1 as a [P,1] broadcast).
    """
    from concourse import dve_ops as dops
    from concourse.dve_spec import (
        C0, C1, C2, C3, Spec, Src0, _has_src1, _spill_c3_to_src1, lower, sq,
    )
    from concourse.dve_uop import DveOpSpec

    NAME = "EXPQ4_ANT"
    for op in dops.OPS:
        if op.name == NAME:
            return op

    w = Src0
    p = ((C0 * w + C1) * w + C2) * w + C3
    body = _spill_c3_to_src1(sq(sq(p)))
    spec = Spec(
        body=body,
        reference=lambda in0, in1, s0, s1, imm2: (
            (((s0 * in0 + s1) * in0 + imm2) * in0 + in1) ** 4
        ),
    )
    row = dops._CUSTOM_DVE_ROW_BASE + len(dops.OPS)
    shas = {}
    for ver in ("v3", "v4"):
        tmp = DveOpSpec(name=NAME, opcode=row, uops=lower(spec, ver=ver),
                        rd1_en=_has_src1(spec))
        shas[ver] = tmp.sha(ver)
    op = dops.DveOp(NAME, spec, subdim=False, uops_sha=shas)
    dops.OPS.append(op)
    dops._SUB_OPCODE_FOR_NAME[NAME] = row
    dops.CUSTOM_DVE_SPECS[NAME] = spec
    return op


def make_mm(nc):
    def mm(out, lhsT, rhs, start, stop):
        nc.tensor.matmul(out, lhsT, rhs, start=start, stop=stop)
    return mm


def build_program() -> bass.Bass:
    exp_op = _register_exp_op()
    nc = bacc.Bacc("TRN2", target_bir_lowering=False, debug=False)

    xq_d = nc.declare_dram_parameter("xq", [DIM, NQ], F16, isOutput=False)
    xkv_d = nc.declare_dram_parameter("xkv", [DIM, N], F16, isOutput=False)
    wq_d = nc.declare_dram_parameter("wq", [DIM, INNER], F16, isOutput=False)
    wkv_d = nc.declare_dram_parameter("wkv", [DIM, 2 * INNER], F16, isOutput=False)
    wo_d = nc.declare_dram_parameter("wo", [INNER, DIM], F16, isOutput=False)
    b_d = nc.declare_dram_parameter("b", [DIM], F32, isOutput=False)
    z_d = nc.declare_dram_parameter("z", [DIM, NQ], F32, isOutput=True)

    with tile.TileContext(nc) as tc:
        mm = make_mm(nc)
        with tc.tile_pool(name="keep", bufs=1) as keep, \
             tc.tile_pool(name="work", bufs=2) as work:

            # ---- persistent tiles ----
            q_t = [keep.tile([P, NQ], F16, tag=f"q{m}", name=f"q{m}") for m in range(2)]
            k_t = [keep.tile([P, N], F16, tag=f"k{m}", name=f"k{m}") for m in range(2)]
            vt_big = keep.tile([P, NJT * VSTRIDE], F16, tag="vt", name="vt")
            y_t = [keep.tile([P, NQ], F16, tag=f"y{m}", name=f"y{m}") for m in range(2)]
            wo_t = [keep.tile([P, DIM], F16, tag=f"wo{m}", name=f"wo{m}") for m in range(2)]
            b_t = keep.tile([P, KT], F32, tag="bias", name="bias")
            ones16 = keep.tile([64, 32], F16, tag="ones16", name="ones16")
            onesP16 = keep.tile([P, 1], F16, tag="onesP16", name="onesP16")
            a0_t = keep.tile([P, 1], F32, tag="a0c", name="a0c")

            nc.vector.memset(ones16[:, :], 1.0)
            nc.vector.memset(onesP16[:, :], 1.0)
            # masked ones: col g = 1.0 on partitions [32g, 32g+32), else 0
            ones4 = keep.tile([P, 4], F16, tag="ones4", name="ones4")
            nc.vector.memset(ones4[:, :], 0.0)
            for g in range(4):
                nc.vector.memset(ones4[g * 32:(g + 1) * 32, g:g + 1], 1.0)
            nc.vector.memset(a0_t[:, :], A0)
            # fp8 staging (whole-tensor ops; per-head slices DMA-gathered)
            m8_a = [keep.tile([P, N], F8, tag=f"m8{m}", name=f"m8{m}") for m in range(2)]
            mres_a = [keep.tile([P, N], F8, tag=f"mr{m}", name=f"mr{m}") for m in range(2)]
            q8_a = [keep.tile([P, NQ], F8, tag=f"q8{m}", name=f"q8{m}") for m in range(2)]
            qres_a = [keep.tile([P, NQ], F8, tag=f"qr{m}", name=f"qr{m}") for m in range(2)]
            ksq_a = [keep.tile([P, N], F16, tag=f"ks{m}", name=f"ks{m}") for m in range(2)]
            qsq_a = [keep.tile([P, NQ], F16, tag=f"qs{m}", name=f"qs{m}") for m in range(2)]
            q2row = keep.tile([H, NQ], F16, tag="q2r", name="q2r")
            c0_a = keep.tile([H, NQ], F8, tag="c0", name="c0")
            c1t = keep.tile([H, NQ], F16, tag="c1t", name="c1t")
            c1f_a = keep.tile([H, NQ], F8, tag="c1f", name="c1f")
            c2f_a = keep.tile([H, NQ], F8, tag="c2f", name="c2f")
            q2c = keep.tile([P, 64], F16, tag="q2c", name="q2c")
            bias_all = keep.tile([P, P], F32, tag="ball", name="ball")

            # ======== Phase A: projections ========
            with tc.tile_pool(name="xw", bufs=1) as xw, \
                 tc.tile_pool(name="pp", bufs=2, space="PSUM") as pp, \
                 tc.tile_pool(name="pk", bufs=1, space="PSUM") as pk:
                wq_t = [xw.tile([P, INNER], F16, tag=f"wq{k}", name=f"wq{k}") for k in range(KT)]
                wkv_t = [xw.tile([P, 2 * INNER], F16, tag=f"wkv{k}", name=f"wkv{k}") for k in range(KT)]
                xq_t = [xw.tile([P, NQ], F16, tag=f"xq{k}", name=f"xq{k}") for k in range(KT)]
                xkv_t = [xw.tile([P, N], F16, tag=f"xkv{k}", name=f"xkv{k}") for k in range(KT)]

                xq_r = xq_d[:].rearrange("(t p) n -> t p n", p=P)
                xkv_r = xkv_d[:].rearrange("(t p) n -> t p n", p=P)
                wq_r = wq_d[:].rearrange("(t p) o -> t p o", p=P)
                wkv_r = wkv_d[:].rearrange("(t p) o -> t p o", p=P)
                for k in range(KT):
                    nc.sync.dma_start(out=xq_t[k][:, :], in_=xq_r[k])
                    nc.sync.dma_start(out=xkv_t[k][:, :], in_=xkv_r[k])
                    nc.sync.dma_start(out=wq_t[k][:, :], in_=wq_r[k])
                    nc.sync.dma_start(out=wkv_t[k][:, :], in_=wkv_r[k])
                wo_r = wo_d[:].rearrange("(t p) o -> t p o", p=P)
                for m in range(2):
                    nc.sync.dma_start(out=wo_t[m][:, :], in_=wo_r[m])
                nc.sync.dma_start(out=b_t[:, :], in_=b_d[:].rearrange("(t p) -> p t", p=P))

                # q projection: (DIM x NQ) -> (INNER x NQ)
                for m in range(2):
                    for n in range(NQ // 512):
                        ps = pp.tile([P, 512], F32, tag="proj", name="proj")
                        for k in range(KT):
                            mm(ps[:, :],
                               wq_t[k][:, m * P:(m + 1) * P],
                               xq_t[k][:, n * 512:(n + 1) * 512],
                               start=(k == 0), stop=(k == KT - 1))
                        nc.vector.tensor_copy(q_t[m][:, n * 512:(n + 1) * 512], ps[:, :])

                # k projection: (DIM x N) -> (INNER x N)   (wkv cols 0:256)
                # PSUM->SBUF copies go through ACT (identity, no table cost);
                # DVE is the tighter engine.
                for m in range(2):
                    for n in range(N // 512):
                        ps = pp.tile([P, 512], F32, tag="proj", name="proj")
                        for k in range(KT):
                            mm(ps[:, :],
                               wkv_t[k][:, m * P:(m + 1) * P],
                               xkv_t[k][:, n * 512:(n + 1) * 512],
                               start=(k == 0), stop=(k == KT - 1))
                        nc.scalar.activation(k_t[m][:, n * 512:(n + 1) * 512],
                                             ps[:, :], AF.Identity, scale=1.0)

                # fp8 staging + batched k2/q2: run during the v projection
                for m in range(2):
                    nc.vector.tensor_scalar_mul(m8_a[m][:, :], k_t[m][:, :], -2.0)
                    nc.vector.scalar_tensor_tensor(mres_a[m][:, :], k_t[m][:, :],
                                                   -2.0, m8_a[m][:, :],
                                                   op0=ALU.mult, op1=ALU.subtract)
                    nc.vector.tensor_copy(q8_a[m][:, :], q_t[m][:, :])
                    nc.vector.tensor_sub(qres_a[m][:, :], q_t[m][:, :], q8_a[m][:, :])
                    nc.gpsimd.tensor_mul(ksq_a[m][:, :], k_t[m][:, :], k_t[m][:, :])
                    nc.gpsimd.tensor_mul(qsq_a[m][:, :], q_t[m][:, :], q_t[m][:, :])

                # k2 for all heads/key-tiles -> ACT bias table [128 j, h*16+jt]
                k2ps = pk.tile([P, P], F32, tag="k2", name="k2")
                for m in range(2):
                    for g in range(4):
                        for jt in range(NJT):
                            mm(k2ps[:, (m * 4 + g) * NJT + jt:(m * 4 + g) * NJT + jt + 1],
                               ksq_a[m][:, jt * P:(jt + 1) * P],
                               ones4[:, g:g + 1],
                               start=True, stop=True)
                nc.vector.tensor_scalar(bias_all[:, :], k2ps[:, :],
                                        S4, S4 * DELTA,
                                        op0=ALU.mult, op1=ALU.add)

                # q2 for all heads: 1-col transpose matmuls; chunk e covers
                # queries {i : i mod 8 == e}, so q2c's flat (partition, chunk)
                # order IS the query order - the row DMA needs no transpose.
                q2ps = pk.tile([P, 64], F32, tag="q2", name="q2")
                for m in range(2):
                    qsv = qsq_a[m][:, :].rearrange("d (j e) -> d e j", e=8)
                    for g in range(4):
                        for e in range(8):
                            col = (m * 4 + g) * 8 + e
                            mm(q2ps[:, col:col + 1],
                               qsv[:, e, :],
                               ones4[:, g:g + 1],
                               start=True, stop=True)
                nc.vector.tensor_copy(q2c[:, :], q2ps[:, :])
                for h in range(H):
                    nc.sync.dma_start(
                        out=q2row[h:h + 1, :],
                        in_=q2c[:, h * 8:(h + 1) * 8])
                # 3-term fp8 split of q2
                nc.vector.tensor_copy(c0_a[:, :], q2row[:, :])
                nc.vector.tensor_sub(c1t[:, :], q2row[:, :], c0_a[:, :])
                nc.vector.tensor_copy(c1f_a[:, :], c1t[:, :])
                nc.vector.tensor_sub(c2f_a[:, :], c1t[:, :], c1f_a[:, :])

                # v^T projection: per key tile jt, (128 j x 256 d), strided into
                # vt_big so each head's 32 columns sit next to its ones column.
                nc.vector.tensor_copy(
                    vt_big[:, :].rearrange("p (a c) -> p a c", c=VTW)[:, :, D:D + 1],
                    onesP16[:, 0:1].to_broadcast((P, P, 1)))
                for jt in range(NJT):
                    ps = pp.tile([P, INNER], F32, tag="vtps", name="vtps")
                    for k in range(KT):
                        mm(ps[:, :],
                           xkv_t[k][:, jt * P:(jt + 1) * P],
                           wkv_t[k][:, INNER:2 * INNER],
                           start=(k == 0), stop=(k == KT - 1))
                    dst = vt_big[:, jt * VSTRIDE:(jt + 1) * VSTRIDE] \
                        .rearrange("p (h c) -> p h c", c=VTW)[:, :, 0:D]
                    src = ps[:, :].rearrange("p (h d) -> p h d", d=D)
                    nc.scalar.activation(dst, src, AF.Identity, scale=1.0)

            # ======== Phase B: attention heads ========
            # dist2 via fp8e4m3 DoubleRow matmuls (0.5 cyc/col) with full
            # error compensation: extra contraction rows are free, so the
            # logical 100-row contraction is
            #   rows  0:32  (m8[d],  q8[d])     m8 = fp8(-2k), q8 = fp8(q)
            #   rows 32:64  (m8[d],  qres[d])   qres = fp8(q - q8)
            #   rows 64:96  (mres[d],q8[d])     mres = fp8(-2k - m8)
            #   rows 96:99  (1, c0/c1f/c2f)     3-term fp8 split of q2
            #   row  99     (0, 0)
            # mapped to 50 DoubleRow partition-pairs: r = half*50 + p.
            # k2[j] stays exact on the ACT sqrt bias port (per-partition).
            with tc.tile_pool(name="pd2", bufs=2, space="PSUM") as pd2, \
                 tc.tile_pool(name="po", bufs=2, space="PSUM") as po:
                KP = 50
                kt8_t = [keep.tile([KP, 2 * N], F8, tag=f"kt8{i}", name=f"kt8{i}")
                         for i in range(2)]
                qt8_t = [keep.tile([KP, 2 * NQ], F8, tag=f"qt8{i}", name=f"qt8{i}")
                         for i in range(2)]
                eq_t = [keep.tile([P, NJT * NQ], F16, tag=f"eq{i}", name=f"eq{i}")
                        for i in range(2)]
                po_s = [keep.tile([VTW, NQ], F32, tag=f"pos{i}", name=f"pos{i}")
                        for i in range(2)]

                # constant rows (B half): p 46:49 ones, p 49 zero.  Engine
                # writes must start at legal partition bases, so stage at
                # partition 0 and DMA into place.
                konst1 = keep.tile([3, N], F8, tag="konst1", name="konst1")
                konst0 = keep.tile([1, N], F8, tag="konst0", name="konst0")
                nc.vector.memset(konst1[:, :], 1.0)
                nc.vector.memset(konst0[:, :], 0.0)
                for i in range(2):
                    nc.sync.dma_start(out=kt8_t[i][46:49, N:2 * N], in_=konst1[:, :])
                    nc.sync.dma_start(out=kt8_t[i][49:50, N:2 * N], in_=konst0[:, :])
                    nc.sync.dma_start(out=qt8_t[i][49:50, NQ:2 * NQ], in_=konst0[:, 0:NQ])

                tail = {}

                def emit_tail(ph, last=False):
                    # deferred normalization of head ph.  The (1,1024) row of
                    # row-sums is reciprocal'd as (128,8) via a scatter DMA
                    # there and back (single-partition recip is ~100x slower).
                    # The 32-row broadcast of the reciprocal row is done by a
                    # log-doubling DMA chain (off the PE); the last head keeps
                    # the PE outer-product since its tail is latency-critical.
                    pmt, pmo, psrc = tail.pop(ph)
                    rs128 = work.tile([P, NQ // P], F32, tag="rs", name="rs")
                    nc.sync.dma_start(out=rs128[:, :], in_=psrc[D:D + 1, :])
                    rr128 = work.tile([P, NQ // P], F16, tag="rr", name="rr")
                    with nc.allow_low_precision(reason="fp16 softmax recip"):
                        nc.vector.reciprocal(rr128[:, :], rs128[:, :])
                    rrow = work.tile([1, NQ], F16, tag="rrow", name="rrow")
                    nc.sync.dma_start(out=rrow[:, :], in_=rr128[:, :])
                    if last:
                        prep = pd2.tile([P, NQ], F32, tag="d2", name="d2")
                        for n in range(NQ // 512):
                            mm(prep[0:D, n * 512:(n + 1) * 512],
                               ones16[0:1, 0:D],
                               rrow[:, n * 512:(n + 1) * 512],
                               start=True, stop=True)
                        nc.vector.tensor_mul(y_t[pmt][pmo:pmo + D, :],
                                             psrc[0:D, :], prep[0:D, :])
                    else:
                        preps = work.tile([D, NQ], F16, tag="preps", name="preps")
                        nc.sync.dma_start(out=preps[0:1, :], in_=rrow[:, :])
                        wseg = 1
                        while wseg < D:
                            nc.sync.dma_start(out=preps[wseg:2 * wseg, :],
                                              in_=preps[0:wseg, :])
                            wseg *= 2
                        nc.vector.tensor_mul(y_t[pmt][pmo:pmo + D, :],
                                             psrc[0:D, :], preps[:, :])

                from contextlib import nullcontext

                def emit_gathers(h):
                    mt, mo = h // 4, (h % 4) * D
                    kt8 = kt8_t[h % 2]
                    qt8 = qt8_t[h % 2]
                    nc.sync.dma_start(out=kt8[0:32, 0:N], in_=m8_a[mt][mo:mo + 32, :])
                    nc.sync.dma_start(out=kt8[32:50, 0:N], in_=m8_a[mt][mo:mo + 18, :])
                    nc.sync.dma_start(out=kt8[0:14, N:2 * N], in_=m8_a[mt][mo + 18:mo + 32, :])
                    nc.sync.dma_start(out=kt8[14:46, N:2 * N], in_=mres_a[mt][mo:mo + 32, :])
                    nc.sync.dma_start(out=qt8[0:32, 0:NQ], in_=q8_a[mt][mo:mo + 32, :])
                    nc.sync.dma_start(out=qt8[32:50, 0:NQ], in_=qres_a[mt][mo:mo + 18, :])
                    nc.sync.dma_start(out=qt8[0:14, NQ:2 * NQ], in_=qres_a[mt][mo + 18:mo + 32, :])
                    nc.sync.dma_start(out=qt8[14:46, NQ:2 * NQ], in_=q8_a[mt][mo:mo + 32, :])
                    nc.sync.dma_start(out=qt8[46:47, NQ:2 * NQ], in_=c0_a[h:h + 1, :])
                    nc.sync.dma_start(out=qt8[47:48, NQ:2 * NQ], in_=c1f_a[h:h + 1, :])
                    nc.sync.dma_start(out=qt8[48:49, NQ:2 * NQ], in_=c2f_a[h:h + 1, :])

                def emit_dist2(h):
                    eq = eq_t[h % 2]
                    kt8v = kt8_t[h % 2][:, :].rearrange("p (two c) -> p two c", two=2)
                    qt8v = qt8_t[h % 2][:, :].rearrange("p (two c) -> p two c", two=2)
                    for jt in range(NJT):
                        psd = pd2.tile([P, NQ], F32, tag="d2", name="d2")
                        for n in range(NQ // 512):
                            nc.tensor.matmul(
                                psd[:, n * 512:(n + 1) * 512],
                                kt8v[:, :, jt * P:(jt + 1) * P],
                                qt8v[:, :, n * 512:(n + 1) * 512],
                                start=True, stop=True, perf_mode=DR)
                        nc.scalar.activation(
                            eq[:, jt * NQ:(jt + 1) * NQ],
                            psd[:, :], AF.Sqrt,
                            bias=bias_all[:, h * NJT + jt:h * NJT + jt + 1],
                            scale=S4)
                        if jt % 4 == 3:
                            # E = p(w)^4 chunk as soon as its sqrt's land
                            cch = jt // 4
                            seg = eq[:, cch * 4 * NQ:(cch + 1) * 4 * NQ]
                            nc.vector._custom_dve(exp_op, out=seg, in0=seg,
                                                  in1=a0_t[:, 0:1],
                                                  s0=A3, s1=A2, imm2=A1)

                def emit_attnv(h):
                    eq = eq_t[h % 2]
                    pso = po.tile([VTW, NQ], F32, tag="o", name="o")
                    for jt in range(NJT):
                        for n in range(NQ // 512):
                            mm(pso[:, n * 512:(n + 1) * 512],
                               vt_big[:, jt * VSTRIDE + h * VTW:
                                      jt * VSTRIDE + (h + 1) * VTW],
                               eq[:, jt * NQ + n * 512:jt * NQ + (n + 1) * 512],
                               start=(jt == 0), stop=(jt == NJT - 1))
                    # SBUF staging frees the po slot (DMA cannot read PSUM;
                    # tensor_mul cannot take two PSUM operands).
                    psrc = po_s[h % 2]
                    nc.vector.tensor_copy(psrc[:, :], pso[:, :])
                    tail[h] = (h // 4, (h % 4) * D, psrc)

                # Head PAIRS: one fp8 block (dist2+sqrt+exp for h, h+1) then
                # one fp16 block (attn@v for both) halves the weight-mode
                # switches to 1 of each per pair.  Tails for heads 0..5 use
                # the DMA-replicated reciprocal (no PE ops, emitted inside
                # the fp8 block is fine); heads 6/7 keep the PE broadcast
                # (fp16) and sit inside the fp16 block - head 6's tail hides
                # behind attn@v(7).
                for hp in range(4):
                    h0, h1 = 2 * hp, 2 * hp + 1
                    prio = tc.high_priority(10000) if hp == 0 else nullcontext()
                    prio.__enter__()
                    emit_gathers(h0)
                    emit_gathers(h1)
                    emit_dist2(h0)
                    if hp >= 1:
                        emit_tail(2 * hp - 2)
                        emit_tail(2 * hp - 1)
                    emit_dist2(h1)
                    prio.__exit__(None, None, None)
                    emit_attnv(h0)
                    emit_attnv(h1)

                emit_tail(H - 2, last=True)
                emit_tail(H - 1, last=True)

            # ======== Phase C: output projection + bias (bias via ACT) ========
            # k=0 accumulations (need only heads 0-3, i.e. y_t[0]) are all
            # emitted first so they overlap the last head's tail latency.
            with tc.tile_pool(name="pz", bufs=1, space="PSUM") as pz:
                z_r = z_d[:].rearrange("(t p) n -> t p n", p=P)
                pzt = []
                for m in range(KT):
                    ps = pz.tile([P, NQ], F32, tag=f"z{m}", name=f"z{m}")
                    pzt.append(ps)
                    for n in range(NQ // 512):
                        mm(ps[:, n * 512:(n + 1) * 512],
                           wo_t[0][:, m * P:(m + 1) * P],
                           y_t[0][:, n * 512:(n + 1) * 512],
                           start=True, stop=False)
                for m in range(KT):
                    ps = pzt[m]
                    for n in range(NQ // 512):
                        mm(ps[:, n * 512:(n + 1) * 512],
                           wo_t[1][:, m * P:(m + 1) * P],
                           y_t[1][:, n * 512:(n + 1) * 512],
                           start=False, stop=True)
                    for n in range(NQ // 512):
                        zt = work.tile([P, 512], F32, tag="ytmp", name="ytmp")
                        nc.scalar.activation(zt[:, :],
                                             ps[:, n * 512:(n + 1) * 512],
                                             AF.Identity,
                                             bias=b_t[:, m:m + 1], scale=1.0)
                        nc.sync.dma_start(out=z_r[m][:, n * 512:(n + 1) * 512],
                                          in_=zt[:, :])

    nc.compile()
    return nc


def make_in_maps(x, w_qkv, w_out, b_out):
    x = np.asarray(x, dtype=np.float32)
    w_qkv = np.asarray(w_qkv, dtype=np.float32)
    w_out = np.asarray(w_out, dtype=np.float32)
    b_out = np.asarray(b_out, dtype=np.float32)
    w_qT = np.ascontiguousarray(w_qkv[0:INNER, :].T).astype(np.float16)
    w_kvT = np.ascontiguousarray(w_qkv[INNER:3 * INNER, :].T).astype(np.float16)
    w_oT = np.ascontiguousarray(w_out.T).astype(np.float16)
    x16 = x.astype(np.float16)
    in_maps = []
    for c in range(8):
        b, half = c // 2, c % 2
        in_maps.append({
            "xq": np.ascontiguousarray(x16[b][:, half * NQ:(half + 1) * NQ]),
            "xkv": np.ascontiguousarray(x16[b]),
            "wq": w_qT,
            "wkv": w_kvT,
            "wo": w_oT,
            "b": b_out,
        })
    return in_maps


def assemble_output(results):
    out = np.empty((B, DIM, N), dtype=np.float32)
    for c in range(8):
        b, half = c // 2, c % 2
        out[b][:, half * NQ:(half + 1) * NQ] = results[c]["z"]
    return out


_prog_cache = {}


def kernel(x, w_qkv, w_out, b_out):
    from concourse.bass_utils import run_bass_kernel_spmd
    if "nc" not in _prog_cache:
        _prog_cache["nc"] = build_program()
    nc = _prog_cache["nc"]
    in_maps = make_in_maps(x, w_qkv, w_out, b_out)
    res = run_bass_kernel_spmd(nc, in_maps, list(range(8)))
    return assemble_output(res.results)
